# revision 5
# baseline (speedup 1.0000x reference)
"""Chamfer loss kernel for Trainium2 (8 NeuronCores, data-parallel over batch).

Problem: a, b: [16, 3, 4096] f32 point clouds (D-major). Per batch:
  d[i, j] = ||pa_i - pb_j||^2 = xx_i + yy_j - 2 a_i . b_j
  loss += sum_i min_j d + sum_j min_i d ; final loss / 16.

Sharding: batch dim 16 -> 2 batches per core on 8 cores. Each core computes
its partial scalar; host sums the 8 partials (the "all-reduce").

Single-orientation streaming (v2). The PE produces d[i, j] tiles once (not
both d and d^T as in v1); both reductions come from the same PSUM data:

  - ACT drains each [128, 2048] PSUM tile to fp16 SBUF, negated (s = -d),
    so every later reduction is a MAX. This is the kernel's wall: every d
    element must exit PSUM through a 1 elem/cyc/lane engine, and only ACT
    and DVE can read PSUM.
  - DVE row side: tensor_scalar(op0=max, op1=max, accum_out=...) is
    copy-class (4x mode on fp16 SBUF operands) and its accum_out is the
    free-axis max -> min_j d for the 128 i's of this tile in ONE pass.
  - DVE col side: in-place tensor_tensor max folds the tile into a
    [128, 4096] fp16 accumulator (2x mode).
  - GPSIMD partition_all_reduce(max) collapses the col accumulator's 128
    partitions at batch end, overlapped with the next batch's main loop.
    (tensor_tensor_reduce would fuse the row side further but crashes the
    exec unit on this stack -- verified NRT_EXEC_UNIT_UNRECOVERABLE.)

The d matrix is produced by the PE via a stacked contraction (as in v1):
  d[i, j] = sum_k L[k, i] * R[k, j],  K = 16
with hi/lo bf16 limb splits of -sqrt(2)*a and +sqrt(2)*b for fp32-grade
precision, plus xx/yy embedded as bf16 limb rows against ones rows.
Limbs and norms are computed in a points-major [128, 32*3] layout (FD=96
per op instead of 4096) and DMA-scattered into the [16, 4096] stacks.
"""

from contextlib import ExitStack

import numpy as np

import concourse.bass as bass
import concourse.bacc as bacc_mod
import concourse.bass_isa as bass_isa
import concourse.mybir as mybir
import concourse.tile as tile

B, D, N = 16, 3, 4096
NCORES = 8
BPC = B // NCORES  # batches per core
P = 128            # partition tile
NJ = 512           # matmul free dim (one PSUM bank of fp32)
HALF = 2048        # drain-tile width (4 PSUM banks)
NIT = N // P       # 32 i-blocks per batch
NT = N // P        # points-per-partition in the points-major layout (32)
K = 16             # stacked contraction rows

F32 = mybir.dt.float32
BF16 = mybir.dt.bfloat16
F16 = mybir.dt.float16
X = mybir.AxisListType.X
MAX = mybir.AluOpType.max
MUL = mybir.AluOpType.mult
SUB = mybir.AluOpType.subtract
SQRT2 = float(np.sqrt(2.0))
NEG_BIG = -1.0e30


def _prep_stacks(nc, io, L, R, ones2, a_src, b_src):
    """Build the K=16 stacks for one batch.

    L rows: [ashi*3, ashi*3, aslo*3, aslo*3](coord-major per group),
            12: xxh, 13: xxl, 14: ones, 15: ones
    R rows: [bshi*3, bslo*3, bshi*3, bslo*3],
            12: ones, 13: ones, 14: yyh, 15: yyl
    Limbs/norms computed points-major (FD=96/32), scattered by DMA.
    """
    for (pref, src, sgn, dst, ngrp, eng) in (
            ("a", a_src, -SQRT2, L, (0, 1, 2, 3), nc.sync),
            ("b", b_src, +SQRT2, R, (0, 2, 1, 3), nc.scalar)):
        # points-major, coord-major load: pt[p, d*NT + t] = src[d, p*NT + t]
        pt = io.tile([P, D * NT], F32, tag=pref + "pt")
        eng.dma_start(
            out=pt[:].rearrange("p (d t) -> p d t", t=NT),
            in_=src.rearrange("d (p t) -> p d t", t=NT))
        hi = io.tile([P, D * NT], BF16, tag=pref + "hi")
        nc.scalar.mul(hi[:], pt[:], sgn)
        lo = io.tile([P, D * NT], BF16, tag=pref + "lo")
        nc.vector.scalar_tensor_tensor(
            out=lo[:], in0=pt[:], scalar=sgn, in1=hi[:], op0=MUL, op1=SUB)
        # squared norms (free-axis reduce over d), split into bf16 limbs
        sq = io.tile([P, D * NT], F32, tag=pref + "sq")
        nc.scalar.square(sq[:], pt[:])
        col = io.tile([P, NT], F32, tag=pref + "col")
        nc.vector.tensor_reduce(
            col[:], sq[:].rearrange("p (d t) -> p t d", t=NT), axis=X,
            op=mybir.AluOpType.add)
        nhi = io.tile([P, NT], BF16, tag=pref + "nhi")
        nc.scalar.copy(nhi[:], col[:])
        nlo = io.tile([P, NT], BF16, tag=pref + "nlo")
        nc.vector.tensor_sub(nlo[:], col[:], nhi[:])

        # scatter coord limbs into stack rows: dst[3g+d, p*NT+t] = limb[p, d*NT+t]
        # ngrp gives the group slots for (hi, hi2, lo, lo2) per side.
        ghi = (ngrp[0], ngrp[1])
        glo = (ngrp[2], ngrp[3])
        for limb, gs in ((hi, ghi), (lo, glo)):
            for g in gs:
                for d in range(D):
                    r = 3 * g + d
                    eng.dma_start(
                        out=dst[r:r + 1, :].rearrange("r (p t) -> r p t", t=NT),
                        in_=limb[:, d * NT:(d + 1) * NT])
        # norm limb rows + ones rows
        nrow = 12 if pref == "a" else 14
        orow = 14 if pref == "a" else 12
        for r, limb in ((nrow, nhi), (nrow + 1, nlo)):
            eng.dma_start(
                out=dst[r:r + 1, :].rearrange("r (p t) -> r p t", t=NT),
                in_=limb[:])
        eng.dma_start(out=dst[orow:orow + 2, :], in_=ones2[:])


def _emit(ctx: ExitStack, tc: tile.TileContext, out_d, a_d, b_d, reps=1):
    nc = tc.nc

    const = ctx.enter_context(tc.tile_pool(name="const", bufs=1))
    io = ctx.enter_context(tc.tile_pool(name="io", bufs=2))
    lab = ctx.enter_context(tc.tile_pool(name="lab", bufs=2))
    drain = ctx.enter_context(tc.tile_pool(name="drain", bufs=4))
    mpool = ctx.enter_context(tc.tile_pool(name="mpool", bufs=2))
    red = ctx.enter_context(tc.tile_pool(name="red", bufs=2))
    outp = ctx.enter_context(tc.tile_pool(name="outp", bufs=1))
    ps = ctx.enter_context(tc.tile_pool(name="ps", bufs=2, space="PSUM"))

    ones128 = const.tile([P, 1], F32)
    nc.vector.memset(ones128[:], 1.0)
    ones2 = const.tile([2, N], BF16)
    nc.vector.memset(ones2[:], 1.0)
    junk = const.tile([P, HALF], F16)
    totalneg = outp.tile([P, 1], F32)
    nc.vector.memset(totalneg[:], 0.0)

    for bi in [i % BPC for i in range(BPC * reps)]:
        L = lab.tile([K, N], BF16, tag="L")
        R = lab.tile([K, N], BF16, tag="R")
        _prep_stacks(nc, io, L, R, ones2, a_d[bi], b_d[bi])

        # col-max accumulators (negated space), one per half
        Mcol = mpool.tile([P, N], F16, tag="Mcol")
        # row maxes: one column per (it, half)
        Rrow = red.tile([P, 2 * NIT], F32, tag="Rrow")

        for it in range(NIT):
            ls = slice(it * P, (it + 1) * P)
            for h in range(2):
                dt = ps.tile([P, HALF], F32, tag="dps")
                for q in range(HALF // NJ):
                    j0 = h * HALF + q * NJ
                    nc.tensor.matmul(
                        dt[:, q * NJ:(q + 1) * NJ],
                        lhsT=L[:, ls],
                        rhs=R[:, j0:j0 + NJ],
                        start=True, stop=True)
                # ACT: drain negated to fp16 SBUF (the PSUM-exit wall)
                s = drain.tile([P, HALF], F16, tag="s")
                nc.scalar.mul(s[:], dt[:], -1.0)
                # DVE col fold: Mcol_h = max(Mcol_h, s)  (2x fp16 TT)
                mh = Mcol[:, h * HALF:(h + 1) * HALF]
                if it == 0:
                    nc.vector.tensor_copy(mh, s[:])
                else:
                    nc.vector.tensor_tensor(out=mh, in0=s[:], in1=mh, op=MAX)
                # DVE row max: copy-class 4x with free-axis max accum
                nc.vector.tensor_scalar(
                    out=junk[:], in0=s[:], scalar1=NEG_BIG, scalar2=None,
                    op0=MAX, op1=MAX,
                    accum_out=Rrow[:, 2 * it + h:2 * it + h + 1])

        # row side: per-row max over the two halves, then sum_i -> [128, 1]
        rmax = red.tile([P, NIT], F32, tag="rmax")
        nc.vector.tensor_reduce(
            rmax[:], Rrow[:].rearrange("p (i h) -> p i h", h=2), axis=X,
            op=MAX)
        rsum = red.tile([P, 1], F32, tag="rsum")
        nc.vector.reduce_sum(rsum[:], rmax[:], axis=X)
        nc.vector.tensor_add(totalneg[:], totalneg[:], rsum[:])

        # col side: partition-max on GPSIMD (overlaps next batch), then
        # re-partition [1, 4096] -> [128, 32] by DMA and free-axis sum.
        par = mpool.tile([P, N], F16, tag="par")
        nc.gpsimd.partition_all_reduce(par[:], Mcol[:], P,
                                       bass_isa.ReduceOp.max)
        colT = red.tile([P, NT], F16, tag="colT")
        nc.gpsimd.dma_start(
            out=colT[:],
            in_=par[0:1, :].rearrange("r (p t) -> r p t", t=NT))
        csum = red.tile([P, 1], F32, tag="csum")
        nc.vector.reduce_sum(csum[:], colT[:], axis=X)
        nc.vector.tensor_add(totalneg[:], totalneg[:], csum[:])

    fin = ps.tile([1, 1], F32, tag="dps")
    nc.tensor.matmul(fin[:], lhsT=ones128[:], rhs=totalneg[:], start=True,
                     stop=True)
    outs = outp.tile([1, 1], F32)
    nc.scalar.mul(outs[:], fin[:], -1.0)
    nc.sync.dma_start(out=out_d[:], in_=outs[:])


def build_nc(reps: int = 1) -> bass.Bass:
    nc = bacc_mod.Bacc("TRN2", target_bir_lowering=False, debug=False)
    a_d = nc.dram_tensor("a", [BPC, D, N], F32, kind="ExternalInput").ap()
    b_d = nc.dram_tensor("b", [BPC, D, N], F32, kind="ExternalInput").ap()
    out_d = nc.dram_tensor("out", [1, 1], F32, kind="ExternalOutput").ap()
    with tile.TileContext(nc) as tc:
        with ExitStack() as ctx:
            _emit(ctx, tc, out_d, a_d, b_d, reps=reps)
    nc.compile()
    return nc


_RUNNER_CACHE: dict = {}


def _make_runner(reps: int = 1):
    """Compile once; return a callable (a, b) -> per-core out array [8,1,1]."""
    import jax
    import concourse.mybir as mb
    from concourse.bass2jax import (_bass_exec_p, install_neuronx_cc_hook,
                                    partition_id_tensor)
    from jax.experimental.shard_map import shard_map
    from jax.sharding import Mesh, PartitionSpec

    install_neuronx_cc_hook()
    nc = build_nc(reps=reps)
    partition_name = (nc.partition_id_tensor.name
                     if nc.partition_id_tensor else None)

    in_names, out_names, out_avals, zero_outs = [], [], [], []
    for alloc in nc.m.functions[0].allocations:
        if not isinstance(alloc, mb.MemoryLocationSet):
            continue
        if not alloc.memorylocations:
            continue
        name = alloc.memorylocations[0].name
        if alloc.kind == "ExternalInput":
            if name != partition_name:
                in_names.append(name)
        elif alloc.kind == "ExternalOutput":
            out_names.append(name)
            shape = tuple(alloc.tensor_shape)
            dtype = mb.dt.np(alloc.dtype)
            out_avals.append(jax.core.ShapedArray(shape, dtype))
            zero_outs.append(np.zeros(shape, dtype))
    n_params = len(in_names)
    all_in_names = in_names + out_names
    if partition_name is not None:
        all_in_names = all_in_names + [partition_name]

    def _body(*args):
        operands = list(args)
        if partition_name is not None:
            operands.append(partition_id_tensor())
        return tuple(_bass_exec_p.bind(
            *operands,
            out_avals=tuple(out_avals),
            in_names=tuple(all_in_names),
            out_names=tuple(out_names),
            lowering_input_output_aliases=(),
            sim_require_finite=True,
            sim_require_nnan=True,
            nc=nc,
        ))

    devices = jax.devices()[:NCORES]
    mesh = Mesh(np.asarray(devices), ("core",))
    n_outs = len(out_names)
    sharded = jax.jit(
        shard_map(_body, mesh=mesh,
                  in_specs=(PartitionSpec("core"),) * (n_params + n_outs),
                  out_specs=(PartitionSpec("core"),) * n_outs,
                  check_rep=False),
        donate_argnums=tuple(range(n_params, n_params + n_outs)),
        keep_unused=True)

    def run(a, b):
        per = {"a": a, "b": b}
        concat_in = [per[nm].reshape(NCORES * BPC, D, N) for nm in in_names]
        concat_zeros = [np.zeros((NCORES * z.shape[0], *z.shape[1:]), z.dtype)
                        for z in zero_outs]
        outs = sharded(*concat_in, *concat_zeros)
        return np.asarray(outs[0])  # [8*1, 1]

    return run


def get_runner(reps: int = 1):
    if reps not in _RUNNER_CACHE:
        _RUNNER_CACHE[reps] = _make_runner(reps)
    return _RUNNER_CACHE[reps]


def kernel(a, b):
    a = np.ascontiguousarray(np.asarray(a, dtype=np.float32))
    b = np.ascontiguousarray(np.asarray(b, dtype=np.float32))
    assert a.shape == (B, D, N) and b.shape == (B, D, N)
    run = get_runner()
    outs = run(a, b)
    return np.float32(float(outs.sum()) / B)
